# revision 63
# baseline (speedup 1.0000x reference)
"""Distributed Trainium2 kernel for the dense transformer block.

Sharding: DP2 (batch) x TP4 (heads). Within a TP group the attention output
O^T is exchanged with two small AllToAll collectives (one per packed head
strip, each overlapped with remaining attention compute), after which every
core owns a contiguous 512-token slice end-to-end: projection, LN2 and the
FFN are fully local and the kernel needs no post-projection ReduceScatter.

Key facts exploited:
  - Source bug (faithful): q, k, v all come from the k-third of qkv, so only
    w_attn[:, D:2D] is needed and S = K K^T is symmetric.
  - Symmetry: exp(S) is only computed for upper-triangle tiles (Scalar engine
    is the exp bottleneck at 1 elem/cycle/lane); lower tiles are produced by
    PE transposes of upper tiles.
  - fp8(e4m3) DoubleRow matmuls (2 k-planes per pass) for kproj/G/proj/fc1/
    fc2; S stays bf16 (K=64 cannot pair). Host-side weight scaling by 32/64
    keeps fp8 values in the normal range; descaled on PSUM read-out.
  - All transposes are PE-transposes (no DRAM round trips), and phase 0 is
    pipelined per 128-token strip so the PE warms up early and stays busy
    (HAM clock gate: idle PE drops 2.4 -> 1.2 GHz).
  - LN gains folded into the following weight matrices on the host; all bias
    vectors are exactly zero (asserted).
"""

import sys

sys.path.insert(0, "/opt/trn_rl_repo")

from contextlib import ExitStack

import ml_dtypes
import numpy as np

import concourse.bass as bass
from concourse import bacc
from concourse import mybir
from concourse.bass import ts
from concourse.bass_utils import run_bass_kernel_spmd
from concourse.masks import make_identity
from concourse.tile import TileContext

F32 = mybir.dt.float32
BF16 = mybir.dt.bfloat16
F8 = mybir.dt.float8e4
NP_BF16 = ml_dtypes.bfloat16
NP_F8 = ml_dtypes.float8_e4m3

B, L, D = 2, 2048, 1024
H = 16          # total heads
DH = 64         # head dim
DFF = 4096
EPS = 1e-5
P = 128

TP = 4          # tensor-parallel group size (heads)
HL = H // TP    # heads per core = 4
C = HL * DH     # per-core k-proj cols = 256
TOK = L // TP   # tokens per core after A2A = 512

LT = L // P     # 16 token tiles
TT = TOK // P   # 4 token tiles per core
EXP_BIAS = -1.5  # global shift inside exp(); cancels in softmax, keeps
                 # e4m3 E values in [~0.06, ~90] (no overflow past 240)

STOP_PHASE = None  # debug: "pre" | "attn" | "proj" | "res" | None


def _dummy_out(nc, tc, out):
    with tc.tile_pool(name="dummy", bufs=1) as pdum:
        z = pdum.tile([P, D], F32, name="z")
        nc.vector.memset(z[:], 0.0)
        for t in range(TT):
            nc.sync.dma_start(out=out[ts(t, P), :], in_=z[:])


def build(nc: bass.Bass):
    xb = nc.declare_dram_parameter("xb", [L, D], BF16, isOutput=False)
    xs = nc.declare_dram_parameter("xs", [TOK, D], F32, isOutput=False)
    wk = nc.declare_dram_parameter("wk", [D, C], F8, isOutput=False)
    # per-core: rows for out-of-group ranks are zero (A2A is global 8-way)
    wproj = nc.declare_dram_parameter("wproj", [2 * D, D], F8, isOutput=False)
    wfc1 = nc.declare_dram_parameter("wfc1", [D, DFF], F8, isOutput=False)
    wfc2 = nc.declare_dram_parameter("wfc2", [DFF, D], F8, isOutput=False)
    out = nc.declare_dram_parameter("out", [TOK, D], F32, isOutput=True)

    with TileContext(nc) as tc, ExitStack() as ctx:
        persist = ctx.enter_context(tc.tile_pool(name="persist", bufs=1))
        pool_scr = ctx.enter_context(tc.tile_pool(name="scratch", bufs=3))
        pool_dram = ctx.enter_context(tc.tile_pool(name="dram", bufs=1, space="DRAM"))

        ident8 = persist.tile([P, P], F8, name="ident8")
        make_identity(nc, ident8)
        identb = persist.tile([P, P], BF16, name="identb")
        make_identity(nc, identb)
        ones_col = persist.tile([1, DH], BF16, name="ones_col")
        nc.vector.memset(ones_col[:], 1.0)
        eps_t = persist.tile([P, 1], F32, name="eps_t")
        nc.vector.memset(eps_t[:], float(EPS))
        neg_b = persist.tile([P, 1], F32, name="neg_b")
        nc.vector.memset(neg_b[:], float(EXP_BIAS))

        # persistent SBUF tensors
        kT = persist.tile([P, 2, L], BF16, name="kT")              # k^T strips
        # V stationary padded to 72 cols: 0-63 = V, 64 = ones (Z row), 65-71
        # pad (dual-fp8 LDW needs even cols / 8B-aligned plane stride)
        DHP = 72
        vpair = persist.tile([P, LT // 2, 2, HL, DHP], F8, name="vpair")
        # one tile per strip so the strip-0 A2A staging DMA does not carry a
        # false dependency on strip-1 writes (DMA read deps are tile-granular)
        ot0 = persist.tile([P, L], F8, name="ot0")
        ot1 = persist.tile([P, L], F8, name="ot1")
        ots = [ot0, ot1]
        res1 = persist.tile([P, TT, D], F32, name="res1")
        xn2T = persist.tile([P, 4, 2, TOK], F8, name="xn2T")
        a2a_sb = persist.tile([P, 8, 2, TOK], F8, name="a2a_sb")
        wk_sb = persist.tile([P, 4, 2, C], F8, name="wk_sb")
        wproj_sb = persist.tile([P, 8, 2, D], F8, name="wproj_sb")
        w1_sb = persist.tile([P, 4, 2, DFF], F8, name="w1_sb")

        nc.sync.dma_start(out=wk_sb[:], in_=wk[:].rearrange("(a b p) c -> p a b c", b=2, p=P))
        nc.sync.dma_start(out=wproj_sb[:], in_=wproj[:].rearrange("(a b p) c -> p a b c", b=2, p=P))
        nc.sync.dma_start(out=w1_sb[:], in_=wfc1[:].rearrange("(a b p) c -> p a b c", b=2, p=P))
        nc.vector.memset(vpair[:], 1.0)

        a2a_in = [pool_dram.tile([8 * P, TOK], F8, name=f"a2a_in{s}") for s in range(2)]
        a2a_out = [pool_dram.tile([8 * P, TOK], F8, name=f"a2a_out{s}")
                   for s in range(2)]

        def ln_stats(x_strip, n):
            """mean/var over free axis; returns (mu_neg, rsq) [P,1] f32.

            var = E[x^2] - mu^2 so the Scalar sum-of-squares pass needs no
            mean and runs concurrently with the DVE mean-reduce."""
            ssum = pool_scr.tile([P, 1], F32, name="ssum", tag="ssum")
            mu_neg = pool_scr.tile([P, 1], F32, name="mu_neg", tag="mu_neg")
            sq = pool_scr.tile([P, n], BF16, name="sq", tag="sq")
            ss = pool_scr.tile([P, 1], F32, name="ss", tag="ss")
            em = pool_scr.tile([P, 1], F32, name="em", tag="em")
            sd = pool_scr.tile([P, 1], F32, name="sd", tag="sd")
            rsq = pool_scr.tile([P, 1], F32, name="rsq", tag="rsq")
            nc.scalar.activation(sq[:], x_strip, mybir.ActivationFunctionType.Square,
                                 accum_out=ss[:])
            nc.vector.tensor_reduce(ssum[:], x_strip, mybir.AxisListType.X, mybir.AluOpType.add)
            nc.vector.tensor_scalar_mul(mu_neg[:], ssum[:], -1.0 / n)
            # em = eps - mu^2
            nc.vector.tensor_tensor(em[:], mu_neg[:], mu_neg[:], mybir.AluOpType.mult)
            nc.vector.tensor_scalar(em[:], em[:], -1.0, float(EPS),
                                    mybir.AluOpType.mult, mybir.AluOpType.add)
            nc.scalar.activation(sd[:], ss[:], mybir.ActivationFunctionType.Sqrt,
                                 bias=em[:], scale=1.0 / n)
            nc.vector.reciprocal(rsq[:], sd[:])
            return mu_neg, rsq

        # ---------------- Phase 0: LN1 + transpose + k projection + V ----------------
        with tc.tile_pool(name="p0", bufs=1) as pool_p0, \
             tc.tile_pool(name="xin", bufs=3) as pool_x, \
             tc.tile_pool(name="pt", bufs=4, space="PSUM") as pt_pool, \
             tc.tile_pool(name="pk", bufs=2, space="PSUM") as pk_pool, \
             tc.tile_pool(name="ptv", bufs=2, space="PSUM") as ptv_pool:

            xn1T = pool_p0.tile([P, 4, 2, L], F8, name="xn1T")

            for t in range(LT):
                x_t = pool_x.tile([P, D], BF16, name="x_t", tag="x")
                nc.sync.dma_start(out=x_t[:], in_=xb[ts(t, P), :])
                mu_neg, rsq = ln_stats(x_t[:], D)
                xn1_t = pool_x.tile([P, D], F8, name="xn1_t", tag="xn1")
                if t % 2 == 0:
                    nc.vector.tensor_scalar(xn1_t[:], x_t[:], mu_neg[:], rsq[:],
                                            mybir.AluOpType.add, mybir.AluOpType.mult)
                else:
                    nb = pool_scr.tile([P, 1], F32, name="nb", tag="nb")
                    nc.vector.tensor_tensor(nb[:], mu_neg[:], rsq[:],
                                            mybir.AluOpType.mult)
                    nc.scalar.activation(xn1_t[:], x_t[:],
                                         mybir.ActivationFunctionType.Identity,
                                         bias=nb[:], scale=rsq[:])
                # fp8 PE transpose writes PSUM with element step 2; batch the
                # 8 tile-transposes into one PSUM tile, drain with 2 copies
                pt = pt_pool.tile([P, D, 2], F8, name="pt", tag="pt")
                for kd in range(8):
                    nc.tensor.transpose(pt[:, ts(kd, P), 0], xn1_t[:, ts(kd, P)],
                                        ident8[:])
                ptv_ = pt[:, :, 0].rearrange("p (a b c) -> p a b c", a=4, b=2)
                nc.vector.tensor_copy(out=xn1T[:, :, 0, ts(t, P)],
                                      in_=ptv_[:, :, 0, :])
                nc.scalar.activation(xn1T[:, :, 1, ts(t, P)], ptv_[:, :, 1, :],
                                     mybir.ActivationFunctionType.Copy)

                if t % 4 == 3:
                    nt = t // 4
                    # kproj for this 512-token chunk (DoubleRow fp8, K=1024)
                    for s in range(2):
                        pk = pk_pool.tile([P, 512], F32, name="pk", tag="pk")
                        for kd2 in range(4):
                            nc.tensor.matmul(pk[:], wk_sb[:, kd2, :, ts(s, P)],
                                             xn1T[:, kd2, :, ts(nt, 512)],
                                             start=(kd2 == 0), stop=(kd2 == 3),
                                             perf_mode=mybir.MatmulPerfMode.DoubleRow)
                        nc.scalar.activation(kT[:, s, ts(nt, 512)], pk[:],
                                             mybir.ActivationFunctionType.Copy,
                                             scale=1.0 / 32)
                    # V tiles: PE-transpose the fresh kT blocks; batch copies
                    for s in range(2):
                        ptv = ptv_pool.tile([P, 4, P], BF16, name="ptv", tag="ptv")
                        for q in range(4):
                            nc.tensor.transpose(ptv[:, q, :],
                                                kT[:, s, ts(nt * 4 + q, P)], identb[:])
                        # psum cols (q, half, d) -> vpair[u2=(4nt+q)//2, j=q%2, head 2s+half, d]
                        pv = ptv[:].rearrange("p (u2 j) (hh d) -> p u2 j hh d",
                                              u2=2, hh=2)
                        nc.vector.tensor_copy(
                            out=vpair[:, nt * 2:nt * 2 + 2, :, 2 * s:2 * s + 2, 0:DH],
                            in_=pv)

        if STOP_PHASE == "pre":
            _dummy_out(nc, tc, out)
            return nc

        # ---------------- Phase 1: attention (symmetric E, upper tiles only) --------
        with tc.tile_pool(name="epool", bufs=2) as pool_e, \
             tc.tile_pool(name="gpool", bufs=2) as pool_g, \
             tc.tile_pool(name="ps_s", bufs=2, space="PSUM") as ps_pool, \
             tc.tile_pool(name="ps_g", bufs=1, space="PSUM") as pg_pool, \
             tc.tile_pool(name="ps_t", bufs=2, space="PSUM") as pt8_pool:

            for h in range(HL):
                s_, r0 = h // 2, (h % 2) * DH
                kh = kT[r0:r0 + DH, s_, :]  # [64, L] bf16
                eps_ = [pool_e.tile([P, 2, L], F8, name=f"ep{u2}", tag=f"ep{u2}")
                        for u2 in range(LT // 2)]
                for t in range(LT):
                    et = eps_[t // 2][:, t % 2, :]
                    # upper-triangle S row-run: cols [128t, 2048)
                    for c0 in range(128 * t, L, 1024):
                        cw = min(1024, L - c0)
                        ps = ps_pool.tile([P, 1024], F32, name="ps", tag="s")
                        for cc in range(0, cw, 512):
                            ccw = min(512, cw - cc)
                            nc.tensor.matmul(ps[:, cc:cc + ccw], kh[:, ts(t, P)],
                                             kh[:, c0 + cc:c0 + cc + ccw],
                                             start=True, stop=True)
                        nc.scalar.activation(et[:, c0:c0 + cw], ps[:, 0:cw],
                                             mybir.ActivationFunctionType.Exp,
                                             bias=neg_b[:], scale=0.125)
                    # lower-triangle tiles: transpose of earlier rows, batched
                    # 8 per PSUM tile and drained with one wide copy each
                    for u0 in range(0, t, 8):
                        n_u = min(8, t - u0)
                        pt8 = pt8_pool.tile([P, 8 * P, 2], F8, name="pt8", tag="t8")
                        for g in range(n_u):
                            u = u0 + g
                            nc.tensor.transpose(pt8[:, ts(g, P), 0],
                                                eps_[u // 2][:, u % 2, ts(t, P)],
                                                ident8[:])
                        nc.vector.tensor_copy(out=et[:, u0 * P:(u0 + n_u) * P],
                                              in_=pt8[:, 0:n_u * P, 0])

                if STOP_PHASE == "edump" and h == 0:
                    with tc.tile_pool(name="edmp", bufs=1) as pdmp:
                        for (r, src) in enumerate([
                                eps_[4][:, 0, 0:1024], eps_[4][:, 0, 1024:2048],
                                eps_[0][:, 0, 0:1024], kT[:, 0, 0:1024]]):
                            df = pdmp.tile([P, 1024], F32, name="df", tag="df")
                            nc.vector.tensor_copy(out=df[:], in_=src)
                            nc.sync.dma_start(out=out[ts(r, P), :], in_=df[:])
                    return nc

                # G' = [V;1]^T E (DoubleRow over token-tile pairs), per q-half
                for qh in range(2 if STOP_PHASE != "vdump" or h > 0 else 1):
                    pg = pg_pool.tile([DHP, 1024], F32, name="pg", tag="g")
                    for u2 in range(LT // 2):
                        for nq in range(2):
                            nc.tensor.matmul(
                                pg[:, ts(nq, 512)], vpair[:, u2, :, h, :],
                                eps_[u2][:, :, qh * 1024 + 512 * nq:qh * 1024 + 512 * (nq + 1)],
                                start=(u2 == 0), stop=(u2 == LT // 2 - 1),
                                perf_mode=mybir.MatmulPerfMode.DoubleRow)
                    g_sb = pool_g.tile([DHP, 1024], F32, name="g_sb", tag="g")
                    nc.vector.tensor_copy(out=g_sb[:], in_=pg[:])
                    zcp = pool_g.tile([1, 1024], F32, name="zcp", tag="zcp")
                    # custom DVE ops ignore the input partition offset -> copy
                    # the Z row down to partition 0 first
                    nc.vector.tensor_copy(out=zcp[:], in_=g_sb[DH:DH + 1, :])
                    zr = pool_g.tile([1, 1024], F32, name="zr", tag="zr")
                    nc.vector.reciprocal_approx_fast(out=zr[:], in_=zcp[:])
                    zr_bf = pool_g.tile([1, 1024], BF16, name="zr_bf", tag="zrb")
                    nc.vector.tensor_copy(out=zr_bf[:], in_=zr[:])
                    pz = pg_pool.tile([DH, 1024], F32, name="pz", tag="g")
                    for nq in range(2):
                        nc.tensor.matmul(pz[:, ts(nq, 512)], ones_col[:],
                                         zr_bf[:, ts(nq, 512)], start=True, stop=True)
                    nc.vector.tensor_tensor(ots[s_][r0:r0 + DH, qh * 1024:(qh + 1) * 1024],
                                            g_sb[0:DH, :], pz[:], mybir.AluOpType.mult)
                    if STOP_PHASE == "vdump" and h == 0 and qh == 0:
                        with tc.tile_pool(name="vdmp", bufs=1) as pdmp:
                            df = pdmp.tile([P, 1024], F32, name="df", tag="df")
                            nc.vector.memset(df[:], 0.0)
                            nc.vector.tensor_copy(out=df[0:DHP, :], in_=g_sb[:])
                            nc.sync.dma_start(out=out[ts(0, P), :], in_=df[:])
                            for r in range(1, 3):
                                # vpair token-tiles 4(r-1)..: [128, 2*2*288? dump u2 pair r-1
                                df2 = pdmp.tile([P, 1024], F32, name="df2", tag="df")
                                nc.vector.memset(df2[:], 0.0)
                                nc.vector.tensor_copy(
                                    out=df2[:, 0:2 * HL * DHP].rearrange(
                                        "p (j h c) -> p j h c", j=2, h=HL),
                                    in_=vpair[:, r - 1, :, :, :])
                                nc.sync.dma_start(out=out[ts(r, P), :], in_=df2[:])
                            df3 = pdmp.tile([P, 1024], F32, name="df3", tag="df")
                            nc.vector.memset(df3[:], 0.0)
                            nc.vector.tensor_copy(out=df3[0:1, :], in_=zr[:])
                            nc.vector.tensor_copy(out=df3[32:33, :], in_=zr_bf[:])
                            nc.vector.tensor_copy(out=df3[DH:2 * DH, :], in_=pz[:])
                            nc.sync.dma_start(out=out[ts(3, P), :], in_=df3[:])
                        return nc

                # strip complete after heads 1 and 3 -> overlap A2A with next heads
                if h % 2 == 1 and STOP_PHASE != "nocc":
                    strip = h // 2
                    for j in range(8):
                        # scalar-engine queue reaches these right after this
                        # head pair's exps -> collective fires ~50us earlier
                        nc.scalar.dma_start(out=a2a_in[strip][ts(j, P), :],
                                            in_=ots[strip][:, ts(j % 4, 512)])
                    nc.gpsimd.collective_compute(
                        "AllToAll", mybir.AluOpType.bypass,
                        replica_groups=[[0, 1, 2, 3, 4, 5, 6, 7]],
                        ins=[a2a_in[strip][:]], outs=[a2a_out[strip][:]])

        if STOP_PHASE == "odump":
            with tc.tile_pool(name="odmp", bufs=1) as pdmp:
                for r in range(4):
                    df = pdmp.tile([P, 1024], F32, name="df", tag="df")
                    nc.vector.tensor_copy(out=df[:], in_=ots[r // 2][:, (r % 2) * 1024:(r % 2 + 1) * 1024])
                    nc.sync.dma_start(out=out[ts(r, P), :], in_=df[:])
            return nc

        if STOP_PHASE in ("attn", "nocc"):
            _dummy_out(nc, tc, out)
            return nc

        # ---------------- Phase 2: local projection + residual + LN2 ----------------
        # strip-0 contraction half starts as soon as A2A#1 lands (overlaps
        # A2A#2); strip-1 half closes each accumulation group afterwards.
        for j in range(2):
            nc.sync.dma_start(out=a2a_sb[:, :, j, :],
                              in_=a2a_out[j][:].rearrange("(i p) t -> p i t", p=P))
        with tc.tile_pool(name="rpool", bufs=4) as pool_r, \
             tc.tile_pool(name="pp", bufs=1, space="PSUM") as pp_pool:
            pps = {}
            for j in range(2):
                for tt in range(TT):
                    for n2 in range(2):
                        if j == 0:
                            pps[(tt, n2)] = pp_pool.tile([P, 512], F32, name="pp",
                                                         tag=f"pp{tt}{n2}")
                        pp = pps[(tt, n2)]
                        for i in range(8):
                            nc.tensor.matmul(pp[:], a2a_sb[:, i, j, ts(tt, P)],
                                             wproj_sb[:, i, j, ts(n2, 512)],
                                             start=(j == 0 and i == 0),
                                             stop=(j == 1 and i == 7))
            for tt in range(TT):
                attn_f = pool_r.tile([P, D], F32, name="attn_f", tag="attn")
                for n2 in range(2):
                    nc.scalar.activation(attn_f[:, ts(n2, 512)], pps[(tt, n2)][:],
                                         mybir.ActivationFunctionType.Copy,
                                         scale=1.0 / 32)
                xs_t = pool_r.tile([P, D], F32, name="xs_t", tag="xs")
                nc.sync.dma_start(out=xs_t[:], in_=xs[ts(tt, P), :])
                nc.vector.tensor_tensor(res1[:, tt, :], xs_t[:], attn_f[:],
                                        mybir.AluOpType.add)
        with tc.tile_pool(name="rpool2", bufs=4) as pool_r2, \
             tc.tile_pool(name="pt2", bufs=2, space="PSUM") as pt2_pool:
            for tt in range(TT):
                mu_neg, rsq = ln_stats(res1[:, tt, :], D)
                xn2_t = pool_r2.tile([P, D], F8, name="xn2_t", tag="xn2")
                nc.vector.tensor_scalar(xn2_t[:], res1[:, tt, :], mu_neg[:], rsq[:],
                                        mybir.AluOpType.add, mybir.AluOpType.mult)
                pt2 = pt2_pool.tile([P, D, 2], F8, name="pt2", tag="pt2")
                for kd in range(8):
                    nc.tensor.transpose(pt2[:, ts(kd, P), 0], xn2_t[:, ts(kd, P)],
                                        ident8[:])
                pv2 = pt2[:, :, 0].rearrange("p (a b c) -> p a b c", a=4, b=2)
                nc.vector.tensor_copy(out=xn2T[:, :, 0, ts(tt, P)],
                                      in_=pv2[:, :, 0, :])
                nc.scalar.activation(xn2T[:, :, 1, ts(tt, P)], pv2[:, :, 1, :],
                                     mybir.ActivationFunctionType.Copy)

        if STOP_PHASE == "adump":
            with tc.tile_pool(name="admp", bufs=1) as pdmp:
                for r in range(4):
                    df = pdmp.tile([P, 1024], F32, name="df", tag="df")
                    nc.vector.tensor_copy(
                        out=df[:].rearrange("p (i t) -> p i t", i=8),
                        in_=a2a_sb[:, :, r // 2, (r % 2) * 128:(r % 2 + 1) * 128])
                    nc.sync.dma_start(out=out[ts(r, P), :], in_=df[:])
            return nc

        if STOP_PHASE == "rdump":
            with tc.tile_pool(name="rdmp", bufs=1) as pdmp:
                for r in range(4):
                    df = pdmp.tile([P, 1024], F32, name="df", tag="df")
                    nc.vector.tensor_copy(out=df[:], in_=res1[:, r, :])
                    nc.sync.dma_start(out=out[ts(r, P), :], in_=df[:])
            return nc

        if STOP_PHASE == "res":
            _dummy_out(nc, tc, out)
            return nc

        # ---------------- Phase 3: FFN (fp8 DoubleRow) ----------------
        with tc.tile_pool(name="ffn", bufs=1) as pool_ffn, \
             tc.tile_pool(name="opool", bufs=2) as pool_o, \
             tc.tile_pool(name="pf", bufs=4, space="PSUM") as pf_pool:
            w2_sb = pool_ffn.tile([P, 16, 2, D], F8, name="w2_sb")
            nc.sync.dma_start(out=w2_sb[:], in_=wfc2[:].rearrange("(a b p) c -> p a b c", b=2, p=P))
            hT = pool_ffn.tile([P, 16, 2, TOK], F8, name="hT")
            for mf in range(32):
                pf = pf_pool.tile([P, TOK], F32, name="pf", tag="pf")
                for kd2 in range(4):
                    nc.tensor.matmul(pf[:], w1_sb[:, kd2, :, ts(mf, P)],
                                     xn2T[:, kd2, :, :],
                                     start=(kd2 == 0), stop=(kd2 == 3),
                                     perf_mode=mybir.MatmulPerfMode.DoubleRow)
                nc.scalar.activation(hT[:, mf // 2, mf % 2, :], pf[:],
                                     mybir.ActivationFunctionType.Relu,
                                     scale=1.0 / 32)
            for tc_ in range(TT):
                po = pf_pool.tile([P, D], F32, name="po", tag="po", bufs=2)
                for n2 in range(2):
                    for kf2 in range(16):
                        nc.tensor.matmul(po[:, ts(n2, 512)],
                                         hT[:, kf2, :, ts(tc_, P)],
                                         w2_sb[:, kf2, :, ts(n2, 512)],
                                         start=(kf2 == 0), stop=(kf2 == 15),
                                         perf_mode=mybir.MatmulPerfMode.DoubleRow)
                ffn_f = pool_o.tile([P, D], F32, name="ffn_f", tag="ffn")
                nc.vector.tensor_scalar_mul(ffn_f[:], po[:], 1.0 / 64)
                out_sb = pool_o.tile([P, D], F32, name="out_sb", tag="out")
                nc.vector.tensor_tensor(out_sb[:], ffn_f[:], res1[:, tc_, :],
                                        mybir.AluOpType.add)
                nc.sync.dma_start(out=out[ts(tc_, P), :], in_=out_sb[:])

    return nc


_CACHE = {}


def _get_nc():
    key = ("nc", STOP_PHASE)
    if key not in _CACHE:
        nc = bacc.Bacc(num_devices=8)
        build(nc)
        if not nc.is_finalized():
            nc.finalize()
        _CACHE[key] = nc
    return _CACHE[key]


def kernel(x, w_attn, b_attn, w_proj, b_proj, ln1_g, ln1_b, ln2_g, ln2_b,
           w_fc1, b_fc1, w_fc2, b_fc2, _trace=False):
    x = np.asarray(x, np.float32)
    for b_ in (np.asarray(b_attn)[D:2 * D], b_proj, b_fc1, b_fc2, ln1_b, ln2_b):
        assert np.abs(np.asarray(b_)).max() == 0.0, "nonzero bias unsupported"

    wk_full = (np.asarray(ln1_g, np.float32)[:, None]
               * np.asarray(w_attn, np.float32)[:, D:2 * D]) * 32.0
    wproj_f = np.asarray(w_proj, np.float32) * 32.0
    # wproj_big[b]: row (i*256 + j*128 + p) multiplies A2A block from rank i,
    # strip j, partition p = head 4i+2j+(p//64), dim p%64 -- zero for ranks
    # outside batch b's TP group.
    wproj_big = []
    for b in range(B):
        wb = np.zeros((2 * D, D), np.float32)
        for i in range(4 * b, 4 * b + 4):
            for j in range(2):
                for half in range(2):
                    head = 4 * (i - 4 * b) + 2 * j + half
                    wb[i * 256 + j * 128 + 64 * half:
                       i * 256 + j * 128 + 64 * half + 64] = \
                        wproj_f[64 * head:64 * head + 64]
        wproj_big.append(np.ascontiguousarray(wb.astype(NP_F8)))
    wfc1_8 = np.ascontiguousarray(
        (np.asarray(ln2_g, np.float32)[:, None]
         * np.asarray(w_fc1, np.float32) * 32.0).astype(NP_F8))
    wfc2_8 = np.ascontiguousarray(
        (np.asarray(w_fc2, np.float32) * 64.0).astype(NP_F8))

    x_bf = [np.ascontiguousarray(x[b].astype(NP_BF16)) for b in range(B)]

    in_maps = []
    for c in range(8):
        tp, b = c % TP, c // TP
        in_maps.append({
            "xb": x_bf[b],
            "xs": np.ascontiguousarray(x[b][TOK * tp:TOK * (tp + 1)]),
            "wk": np.ascontiguousarray(wk_full[:, tp * C:(tp + 1) * C].astype(NP_F8)),
            "wproj": wproj_big[b],
            "wfc1": wfc1_8,
            "wfc2": wfc2_8,
        })

    nc = _get_nc()
    res = run_bass_kernel_spmd(nc, in_maps, core_ids=list(range(8)), trace=_trace)
    results = res.results if hasattr(res, "results") else res

    out = np.empty((B, L, D), np.float32)
    for c in range(8):
        tp, b = c % TP, c // TP
        out[b, TOK * tp:TOK * (tp + 1)] = results[c]["out"]
    if _trace:
        return out, res
    return out
